# revision 29
# baseline (speedup 1.0000x reference)
"""Causal MHA (shared q_linear) Bass kernel for 8 TRN2 NeuronCores.

Sharding: core c handles batch b=c//2, head-group g=c%2 (8 of 16 heads,
columns 512g:512g+512 of the shared projection).  Each core computes a
partial output (its head-group's contribution through Wo); the host sums
the two partials per batch and adds bo.

Design (v15/vbf):
  - x is pre-transposed on the HOST (xT [D, S]): zero PE transposes.
  - bf16 staging for x, Wq, Wo, q, k, v, exp(scores) and attnT (PE
    matmuls run 1 cycle/row either way; DMA bytes halve; bf16 matmuls
    get the split Ldweights+Matmult form whose weight loads pipeline).
  - causal mask applied on DVE: exp of the unmasked diagonal block,
    then one multiply by a 0/1 triangle (keeps the PE stream free of
    mask matmuls and trineg stationary reloads).
  - ACT (scalar engine) does the exp (ONE fused [128,2,512] activation
    per (head-pair, k-tile): in-stream ACT is instruction-bound, so
    fewer/wider beats narrower/faster-per-instr) plus the normalize
    drains.  NO GPSIMD anywhere: gpsimd ops are Q7 ucode with us-scale
    in-stream cost on HW (a partition_broadcast-based normalize chain
    measured ~20us/head-pair vs ~1.6us for the PE-broadcast design).
  - one software pipeline over (head-pair, k-tile): scores+exp run
    `depth` tiles ahead of the accumulating AV matmuls.
  - ACT-free PE work (Wo of the previous Q block, projection groups of
    later token blocks) is interleaved INTO the attention stream as
    "filler" tasks, placed at head-pair boundaries (absorbing the acc
    PSUM handoff) and spread to cover exp backlog.  Wo(Q) runs inside
    attention(Q+1); Wo(3) is the tail.
  - weights/x arrive in 256KB chunks, wq interleaved with the first x
    tensor, so the first matmul starts ~1.5us after kernel entry.

Per-core compute (S=2048, D=1024, 8 local heads of hd=64):
  qT/kT[dt]  = Wq_dt^T @ xT + bq    [128, 4, 512] bf16   (dl on partitions)
  v          = xT^T @ Wq + bq       [128, 4, 8, 65] bf16, ones column fused
  scoresT    = kh @ qh^T per head   PSUM [128, 2, 512] fp32 (+ mask matmul)
  et         = exp(scoresT/8)       bf16
  attnT      = [vh|1]^T @ et        PSUM accum over k; row 64 = sum(exp)
  normalize  : ACT drains acc->SBUF bf16 (frees acc banks early) and
               pulls the sum-of-exp rows; PE broadcasts the sum rows
               (ones-column matmuls into one PSUM bank); DVE
               reciprocal_approx_fast + 2 partition-aligned muls.
               HW gotchas honored: custom-DVE ops only at base
               partition 0 full-width; DVE ops never shift partitions
               (ACT can); SB-SB tensor_tensor inputs base-aligned.
  out        = attnT^T @ Wo         fp32 partial, host adds head-groups + bo
"""

import sys

sys.path.insert(0, "/opt/trn_rl_repo")

import collections

import numpy as np
import concourse.bass as bass  # noqa: F401
import concourse.tile as tile
from concourse import bacc, mybir
from concourse.bass_utils import run_bass_kernel_spmd

F32 = mybir.dt.float32
F32R = mybir.dt.float32r
BF16 = mybir.dt.bfloat16
F16 = mybir.dt.float16
AF = mybir.ActivationFunctionType

S = 2048          # tokens
D = 1024          # model dim
DL = 512          # local (per-core) projection columns = 8 heads * 64
HD = 64           # head dim
NHL = 8           # local heads
TB = 4            # token blocks of 512
JD = 8            # Din blocks of 128
NEG = -1.0e10


def build(repeat: int = 1, mode: str = "full", variant: str = "v15"):
    depth = 3
    qk_dt = BF16 if variant == "vbf" else F32R
    nc = bacc.Bacc("TRN2", target_bir_lowering=False, debug=False)
    x_aps = {
        n: nc.dram_tensor(n, [D, S], BF16, kind="ExternalInput").ap()
        for n in ("xt_q", "xt_k", "xt_v")
    }
    wq_ap = nc.dram_tensor("wq", [D, DL], BF16, kind="ExternalInput").ap()
    bq_ap = nc.dram_tensor("bq", [DL], F32, kind="ExternalInput").ap()
    wo_ap = nc.dram_tensor("wo", [DL, D], BF16, kind="ExternalInput").ap()
    tri2_ap = nc.dram_tensor("tri2", [128, 256], BF16, kind="ExternalInput").ap()
    out_ap = nc.dram_tensor("out", [S, D], F32, kind="ExternalOutput").ap()

    with tile.TileContext(nc) as tc:
        with tc.tile_pool(name="const", bufs=1) as const, \
             tc.tile_pool(name="persist", bufs=1) as persist, \
             tc.tile_pool(name="xt", bufs=10) as xtp, \
             tc.tile_pool(name="exp", bufs=5) as ep, \
             tc.tile_pool(name="at", bufs=2) as atp, \
             tc.tile_pool(name="norm", bufs=2) as normp, \
             tc.tile_pool(name="ob", bufs=4) as obp, \
             tc.tile_pool(name="psS", bufs=2, space="PSUM") as psS, \
             tc.tile_pool(name="psA", bufs=2, space="PSUM") as psA, \
             tc.tile_pool(name="psP", bufs=2, space="PSUM") as psP:

            # ---- weights: wq in 4 chunks, interleaved with the first x
            # tensor's chunks so the first projection matmul starts early
            wqs = [persist.tile([128, 2, DL], BF16, name=f"wq{jj}", tag=f"wq{jj}")
                   for jj in range(4)]
            wq_r4 = wq_ap.rearrange("(jj j p) d -> jj p j d", jj=4, p=128)
            xk0_tiles = [xtp.tile([128, 2, 512], BF16, tag="xt", name=f"xk0_{jj}")
                         for jj in range(4)]
            xq0_tiles = [xtp.tile([128, 2, 512], BF16, tag="xt", name=f"xq0_{jj}")
                         for jj in range(4)]
            xk0_r4 = x_aps["xt_k"].rearrange(
                "(jj j p) (b t) -> jj p j b t", jj=4, p=128, t=512
            )
            xq0_r4 = x_aps["xt_q"].rearrange(
                "(jj j p) (b t) -> jj p j b t", jj=4, p=128, t=512
            )
            for jj in range(4):
                nc.sync.dma_start(wqs[jj][:], wq_r4[jj])
                nc.sync.dma_start(xk0_tiles[jj][:], xk0_r4[jj, :, :, 0, :])
                nc.sync.dma_start(xq0_tiles[jj][:], xq0_r4[jj, :, :, 0, :])
            tri2 = const.tile([128, 256], BF16)
            nc.sync.dma_start(tri2[:], tri2_ap[:])
            ones1 = const.tile([1, 128], BF16)
            nc.vector.memset(ones1[:], 1.0)
            bq_sb = const.tile([128, 4], F32)
            nc.sync.dma_start(bq_sb[:], bq_ap.rearrange("(t p) -> p t", p=128))
            bq_row = const.tile([1, DL], F32)
            nc.sync.dma_start(bq_row[:], bq_ap.rearrange("(a n) -> a n", a=1))
            bq_bc = const.tile([128, DL], F32)
            nc.gpsimd.partition_broadcast(bq_bc[:], bq_row[0:1, :])

            wo_r = persist.tile([128, 4, D], BF16)

            # persistent per-token-block tensors
            kT = [persist.tile([128, 4, 512], qk_dt, name=f"kT{i}", tag=f"kT{i}")
                  for i in range(TB)]
            qT = [persist.tile([128, 4, 512], qk_dt, name=f"qT{i}", tag=f"qT{i}")
                  for i in range(TB)]
            vv = [persist.tile([128, 4, NHL, HD + 1], BF16, name=f"vv{i}",
                               tag=f"vv{i}") for i in range(TB)]
            for i in range(TB):
                nc.vector.memset(vv[i][:, :, :, HD], 1.0)

            # ---------- phase-1 task factories ----------
            def x_dma(tb, name, premade=None):
                """DMA one xt_{k,v,q} tensor for one token block in 4 chunks;
                returns the chunk-tile list."""
                if premade is not None:
                    return premade
                xts = [xtp.tile([128, 2, 512], BF16, tag="xt",
                                name=f"x_{name}{tb}_{jj}") for jj in range(4)]
                x_r4 = x_aps[name].rearrange(
                    "(jj j p) (b t) -> jj p j b t", jj=4, p=128, t=512
                )
                for jj in range(4):
                    nc.sync.dma_start(xts[jj][:], x_r4[jj, :, :, tb, :])
                return xts

            def qk_groups(tb, xk, xq):
                """k and q projections fused per dt-block: the j-loop
                interleaves both so consecutive matmuls share the wq
                stationary (halves PE weight loads)."""
                def grp(i):
                    pk = psP.tile([128, 512], F32, tag="pp", name=f"pk{tb}_{i}")
                    pq = psP.tile([128, 512], F32, tag="pp", name=f"pq{tb}_{i}")
                    for j in range(JD):
                        for ps_, xts in ((pk, xk), (pq, xq)):
                            nc.tensor.matmul(
                                ps_[:],
                                wqs[j // 2][:, j % 2, i * 128:(i + 1) * 128],
                                xts[j // 2][:, j % 2, :],
                                start=(j == 0),
                                stop=(j == JD - 1),
                            )
                    nc.vector.tensor_scalar_add(
                        kT[tb][:, i, :], pk[:], bq_sb[:, i:i + 1]
                    )
                    nc.vector.tensor_scalar_add(
                        qT[tb][:, i, :], pq[:], bq_sb[:, i:i + 1]
                    )

                return [(lambda i=i: grp(i)) for i in range(4)]

            def v_groups(tb, xv):
                def grp(i):
                    pv = psP.tile([128, 512], F32, tag="pp", name=f"pv{tb}_{i}")
                    for j in range(JD):
                        nc.tensor.matmul(
                            pv[:],
                            xv[j // 2][:, j % 2, i * 128:(i + 1) * 128],
                            wqs[j // 2][:, j % 2, :],
                            start=(j == 0),
                            stop=(j == JD - 1),
                        )
                    nc.vector.tensor_add(
                        vv[tb][:, i, :, 0:HD],
                        pv[:].rearrange("p (h d) -> p h d", h=NHL),
                        bq_bc[:].rearrange("p (h d) -> p h d", h=NHL),
                    )

                return [(lambda i=i: grp(i)) for i in range(4)]

            # ---------- Wo task factory ----------
            def wo_groups(Q, attnTs):
                """One group per 128-token sub-block; both 512-wide output
                halves accumulate together so consecutive matmuls share the
                attnT stationary (halves PE weight loads)."""
                def grp(st_):
                    ob = obp.tile([128, D], F32, tag="ob", name=f"ob{Q}_{st_}")
                    po0 = psP.tile([128, 512], F32, tag="pp", name=f"po0_{st_}")
                    po1 = psP.tile([128, 512], F32, tag="pp", name=f"po1_{st_}")
                    for kt in range(4):
                        for nh, po in ((0, po0), (1, po1)):
                            nc.tensor.matmul(
                                po[:],
                                attnTs[kt][:, st_ * 128:(st_ + 1) * 128],
                                wo_r[:, kt, nh * 512:(nh + 1) * 512],
                                start=(kt == 0),
                                stop=(kt == 3),
                            )
                    nc.vector.tensor_copy(ob[:, 0:512], po0[:])
                    nc.vector.tensor_copy(ob[:, 512:1024], po1[:])
                    r0 = Q * 512 + st_ * 128
                    nc.sync.dma_start(out_ap[r0:r0 + 128, :], ob[:])

                return [(lambda st_=st_: grp(st_)) for st_ in range(4)]

            def wo_split_groups(Q, attnTs):
                """Wo in two waves: kt 0..2 accumulated into SBUF (legal as a
                late attention(Q) filler once head-pairs 0..2 are
                normalized), then a short kt=3 + add + store tail."""
                obs = {}

                def partial(st_):
                    ob = obp.tile([128, D], F32, tag="ob", name=f"obp{Q}_{st_}")
                    obs[st_] = ob
                    po0 = psP.tile([128, 512], F32, tag="pp", name=f"pp0_{st_}")
                    po1 = psP.tile([128, 512], F32, tag="pp", name=f"pp1_{st_}")
                    for kt in range(3):
                        for nh, po in ((0, po0), (1, po1)):
                            nc.tensor.matmul(
                                po[:],
                                attnTs[kt][:, st_ * 128:(st_ + 1) * 128],
                                wo_r[:, kt, nh * 512:(nh + 1) * 512],
                                start=(kt == 0),
                                stop=(kt == 2),
                            )
                    nc.vector.tensor_copy(ob[:, 0:512], po0[:])
                    nc.vector.tensor_copy(ob[:, 512:1024], po1[:])

                def final(st_):
                    ob = obs[st_]
                    po0 = psP.tile([128, 512], F32, tag="pp", name=f"pf0_{st_}")
                    po1 = psP.tile([128, 512], F32, tag="pp", name=f"pf1_{st_}")
                    for nh, po in ((0, po0), (1, po1)):
                        nc.tensor.matmul(
                            po[:],
                            attnTs[3][:, st_ * 128:(st_ + 1) * 128],
                            wo_r[:, 3, nh * 512:(nh + 1) * 512],
                            start=True,
                            stop=True,
                        )
                    nc.vector.tensor_add(ob[:, 0:512], ob[:, 0:512], po0[:])
                    nc.vector.tensor_add(ob[:, 512:1024], ob[:, 512:1024], po1[:])
                    r0 = Q * 512 + st_ * 128
                    nc.sync.dma_start(out_ap[r0:r0 + 128, :], ob[:])

                return ([(lambda st_=st_: partial(st_)) for st_ in range(4)],
                        [(lambda st_=st_: final(st_)) for st_ in range(4)])

            # ---------- attention ----------
            def attention(Q, fillers=(), late=(), attnT_pre=None):
                """fillers: list of (closure, max_idx|None); late: closures
                placed in the final stretch of the stream.  Returns the 4
                normalized attnT tiles (Wo is the caller's business)."""
                attnT = attnT_pre if attnT_pre is not None else [
                    atp.tile([128, 512], BF16, tag=f"at{i}",
                             name=f"attnT{Q}_{i}") for i in range(4)]
                qtile = qT[Q]
                nj = 4 * (Q + 1)
                T = 4 * nj
                accs = {}
                srs = {}

                # schedule fillers: constrained ones early (before their
                # deadline), then one per head-pair boundary (absorbs the acc
                # PSUM handoff), rest spread evenly.
                # candidate positions: head-pair boundaries (absorb the acc
                # PSUM handoff) + an even spread; assigned to fillers IN
                # ORDER (list order is execution order), clamped to each
                # filler's deadline.
                sched = collections.defaultdict(list)
                if fillers:
                    n = len(fillers)
                    positions = set(hp * nj + depth for hp in range(1, 4))
                    step = max(1, T // (n + 1))
                    p = step
                    while len(positions) < n:
                        positions.add(min(p, T - 1))
                        p += step
                        if p > 4 * T:
                            break
                    positions = sorted(positions)[:n]
                    while len(positions) < n:
                        positions.append(T - 1)
                    for (f, mx), pos in zip(fillers, positions):
                        sched[min(pos, mx) if mx is not None else pos].append(f)
                for i, f in enumerate(late):
                    sched[T - 1 - 3 * (len(late) - 1 - i)].append(f)

                def emit_scores(hp, j):
                    tbj, sub = j // 4, j % 4
                    qoff = max(0, j * 128 - Q * 512)
                    diag = j * 128 >= Q * 512
                    ps = psS.tile([128, 2, 512], F32, tag="sc", name=f"ps{hp}_{j}")
                    for hi in range(2):
                        nc.tensor.matmul(
                            ps[:, hi, qoff:],
                            kT[tbj][64 * hi:64 * hi + 64, hp,
                                    sub * 128:(sub + 1) * 128],
                            qtile[64 * hi:64 * hi + 64, hp, qoff:],
                            start=True,
                            stop=True,
                        )
                    et = ep.tile([128, 2, 512], BF16, tag="exp", name=f"et{hp}_{j}")
                    if variant == "vsplit":
                        for hi in range(2):
                            nc.scalar.activation(
                                et[:, hi, qoff:], ps[:, hi, qoff:], AF.Exp,
                                scale=0.125
                            )
                    elif variant == "vqhalf" and qoff < 256:
                        nc.scalar.activation(
                            et[:, :, qoff:256], ps[:, :, qoff:256], AF.Exp,
                            scale=0.125
                        )
                        nc.scalar.activation(
                            et[:, :, 256:], ps[:, :, 256:], AF.Exp, scale=0.125
                        )
                    else:
                        nc.scalar.activation(
                            et[:, :, qoff:], ps[:, :, qoff:], AF.Exp, scale=0.125
                        )
                    if diag:
                        # zero the masked (k > q) triangle of the diagonal
                        # 128x128 block on DVE: one mul by a 0/1 triangle,
                        # replicated per head.  Keeps the PE stream free of
                        # mask matmuls and trineg stationary reloads.
                        nc.vector.tensor_mul(
                            et[:, :, qoff:qoff + 128],
                            et[:, :, qoff:qoff + 128],
                            tri2[:].rearrange("p (h c) -> p h c", h=2),
                        )
                    return et

                def emit_attn(hp, j, et):
                    tbj, sub = j // 4, j % 4
                    qoff = max(0, j * 128 - Q * 512)
                    if j == 0:
                        acc0 = psA.tile([128, 512], F32, tag="acc",
                                        name=f"acc0_{hp}")
                        acc1 = psA.tile([128, 512], F32, tag="acc",
                                        name=f"acc1_{hp}")
                        accs[hp] = (acc0, acc1)
                    for hi in range(2):
                        nc.tensor.matmul(
                            accs[hp][hi][0:65, qoff:],
                            vv[tbj][:, sub, hp * 2 + hi, :],
                            et[:, hi, qoff:],
                            start=(j == 0),
                            stop=(j == nj - 1),
                        )

                def norm_a(hp):
                    # ACT drains acc PSUM -> ONE [128,512] bf16 SBUF tile
                    # (head1 lands on partitions 64-127) + the two sum-of-exp
                    # rows as base-0 bf16 rows for the PE broadcast.  Frees
                    # both acc banks ~1.5us after the accumulation stops.
                    ad = normp.tile([128, 512], BF16, tag="ad",
                                    name=f"ad_{hp}")
                    for hi in range(2):
                        nc.scalar.activation(
                            ad[64 * hi:64 * hi + 64, :],
                            accs[hp][hi][0:64, :], AF.Copy
                        )
                        sr = normp.tile([1, 512], BF16, tag=f"sr{hi}",
                                        name=f"sr{hi}_{hp}")
                        nc.scalar.activation(
                            sr[:], accs[hp][hi][64:65, :], AF.Copy
                        )
                        srs[(hp, hi)] = sr
                    srs[(hp, "ad")] = ad

                def norm_b(hp):
                    # PE broadcasts both sum rows into one PSUM bank; DVE
                    # reciprocal_approx_fast (18-bit, plenty for bf16 attnT)
                    # + 2 partition-aligned muls.  No GPSIMD anywhere.
                    ad = srs.pop((hp, "ad"))
                    bc = psP.tile([128, 512], F32, tag="pp", name=f"bc_{hp}")
                    nc.tensor.matmul(bc[0:64, :], ones1[0:1, 0:64],
                                     srs.pop((hp, 0))[:], start=True, stop=True)
                    nc.tensor.matmul(bc[64:128, :], ones1[0:1, 64:128],
                                     srs.pop((hp, 1))[:], start=True, stop=True,
                                     skip_group_check=True)
                    rb = normp.tile([128, 512], F32, tag="rb", name=f"rb_{hp}")
                    nc.vector.reciprocal_approx_fast(rb[:], bc[:])
                    for hi in range(2):
                        nc.vector.tensor_mul(
                            attnT[hp][64 * hi:64 * hi + 64, :],
                            ad[64 * hi:64 * hi + 64, :],
                            rb[64 * hi:64 * hi + 64, :],
                        )

                tasks = [(hp, j) for hp in range(4) for j in range(nj)]
                ets = {}
                normb_pend = {}
                LAG = 3

                def finish(vidx):
                    phl, jl = tasks[vidx - depth]
                    emit_attn(phl, jl, ets.pop((phl, jl)))
                    if jl == nj - 1:
                        norm_a(phl)
                        normb_pend[vidx + LAG] = phl

                for idx, (hp, j) in enumerate(tasks):
                    ets[(hp, j)] = emit_scores(hp, j)
                    for f in sched.get(idx, ()):
                        f()
                    if idx >= depth:
                        finish(idx)
                    if idx in normb_pend:
                        norm_b(normb_pend.pop(idx))
                for idx in range(len(tasks), len(tasks) + depth):
                    finish(idx)
                    if idx in normb_pend:
                        norm_b(normb_pend.pop(idx))
                for idx in sorted(normb_pend):
                    norm_b(normb_pend.pop(idx))
                for idx in sorted(sched):
                    if idx >= T:
                        for f in sched[idx]:
                            f()
                return attnT

            # ---------- orchestration ----------
            def emit_full(rep, tail=()):
                # tb0: k/q chunks were pre-DMA'd interleaved with wq.
                # `tail` is the previous rep's Wo(3) kt=3 wave: emitting it
                # here (between this rep's first projection groups) lets the
                # projections absorb its attnT3/obs dependency latency
                # instead of stalling the PE FIFO at the rep boundary.
                xk0 = x_dma(0, "xt_k", premade=xk0_tiles if rep == 0 else None)
                xq0 = x_dma(0, "xt_q", premade=xq0_tiles if rep == 0 else None)
                tail = list(tail)
                for gi, g in enumerate(qk_groups(0, xk0, xq0)):
                    g()
                    if gi < len(tail):
                        tail[gi]()
                for g in tail[4:]:
                    g()
                if rep == 0:
                    nc.sync.dma_start(
                        wo_r[:], wo_ap.rearrange("(k p) d -> p k d", p=128)
                    )
                xv0 = x_dma(0, "xt_v")
                for g in v_groups(0, xv0):
                    g()
                # tb1 as a block
                xk1 = x_dma(1, "xt_k")
                xq1 = x_dma(1, "xt_q")
                for g in qk_groups(1, xk1, xq1):
                    g()
                xv1 = x_dma(1, "xt_v")
                for g in v_groups(1, xv1):
                    g()
                # attention(0) with the first 2 qk groups of tb2 as fillers
                xk2 = x_dma(2, "xt_k")
                xq2 = x_dma(2, "xt_q")
                gqk2 = qk_groups(2, xk2, xq2)
                at0 = attention(0, fillers=[(g, None) for g in gqk2[:2]])
                for g in gqk2[2:]:
                    g()
                xv2 = x_dma(2, "xt_v")
                for g in v_groups(2, xv2):
                    g()
                # attention(1) with Wo(0) as fillers
                at1 = attention(1, fillers=[(g, None) for g in wo_groups(0, at0)])
                # attention(2): Wo(1) + tb3 k/q projections as fillers
                xk3 = x_dma(3, "xt_k")
                xq3 = x_dma(3, "xt_q")
                at2 = attention(
                    2,
                    fillers=[(g, None) for g in wo_groups(1, at1)]
                    + [(g, None) for g in qk_groups(3, xk3, xq3)],
                )
                # attention(3): tb3-v constrained before first use (v-sub
                # needed at task idx ~12+sub), Wo(2) unconstrained
                xv3 = x_dma(3, "xt_v")
                gv3 = v_groups(3, xv3)
                fill3 = [(gv3[0], 4), (gv3[1], 7), (gv3[2], 9), (gv3[3], 11)]
                # Wo(2) goes in the LATE list (positions ~42..51): the exp
                # backlog deficit peaks there and the generic spread left
                # that window empty (6.4us PE gap).  Wo(3) kt0..2 partials
                # follow at 54..63; only the kt=3 wave remains for the tail.
                at3_tiles = [atp.tile([128, 512], BF16, tag=f"at{i}",
                                      name=f"attnT3_{i}") for i in range(4)]
                part3, final3 = wo_split_groups(3, at3_tiles)
                late3 = wo_groups(2, at2) + part3
                attention(3, fillers=fill3, late=late3, attnT_pre=at3_tiles)
                return final3

            def emit_p1_block(rep):
                for tb in range(TB):
                    first = rep == 0 and tb == 0
                    xk = x_dma(tb, "xt_k", premade=xk0_tiles if first else None)
                    xq = x_dma(tb, "xt_q", premade=xq0_tiles if first else None)
                    for g in qk_groups(tb, xk, xq):
                        g()
                    xv = x_dma(tb, "xt_v")
                    for g in v_groups(tb, xv):
                        g()

            pending_tail = ()
            for rep in range(repeat):
                if mode == "full":
                    pending_tail = emit_full(rep, pending_tail)
                else:
                    # simple un-interleaved paths for microbenchmarks
                    if rep == 0:
                        nc.sync.dma_start(
                            wo_r[:], wo_ap.rearrange("(k p) d -> p k d", p=128)
                        )
                    if mode == "p1":
                        emit_p1_block(rep)
                    elif mode == "attn":
                        if rep == 0:
                            emit_p1_block(0)
                        for Q in range(TB):
                            atq = attention(Q)
                            for g in wo_groups(Q, atq):
                                g()
                    elif mode == "attn3":
                        if rep == 0:
                            emit_p1_block(0)
                        attention(3)
                    elif mode == "attn3wo":
                        if rep == 0:
                            emit_p1_block(0)
                        atq = attention(3)
                        for g in wo_groups(3, atq):
                            g()
                    elif mode == "attn01":
                        if rep == 0:
                            emit_p1_block(0)
                        for Q in (0, 1):
                            atq = attention(Q)
                            for g in wo_groups(Q, atq):
                                g()
            for g in pending_tail:
                g()

    nc.compile()
    return nc


_BUILD_CACHE = {}


def _get(repeat=1, mode="full", variant="v15"):
    key = (repeat, mode, variant)
    if key not in _BUILD_CACHE:
        _BUILD_CACHE[key] = build(repeat, mode, variant)
    return _BUILD_CACHE[key]


def make_in_maps(q, k, v, Wq, bq, Wo, bo, variant="v15"):
    import ml_dtypes
    bf16 = ml_dtypes.bfloat16
    # tri01[k, q] = 1 if k <= q else 0 (causal keep-mask for the diagonal
    # 128x128 block of scoresT), replicated twice along the free axis so one
    # DVE op covers both heads of a pair.
    tri01 = (np.arange(128)[:, None] <= np.arange(128)[None, :]).astype(np.float32)
    tri2 = np.concatenate([tri01, tri01], axis=1).astype(bf16)
    in_maps = []
    for c in range(8):
        b, g = c // 2, c % 2
        sl = slice(g * DL, (g + 1) * DL)
        in_maps.append({
            "xt_q": np.ascontiguousarray(q[b].T).astype(bf16),
            "xt_k": np.ascontiguousarray(k[b].T).astype(bf16),
            "xt_v": np.ascontiguousarray(v[b].T).astype(bf16),
            "wq": np.ascontiguousarray(Wq[:, sl]).astype(bf16),
            "bq": np.ascontiguousarray(bq[sl]),
            "wo": np.ascontiguousarray(Wo[sl, :]).astype(bf16),
            "tri2": tri2,
        })
    return in_maps


DEFAULT_VARIANT = "vbf"


def kernel(q, k, v, Wq, bq, Wo, bo):
    q, k, v, Wq, bq, Wo, bo = (
        np.asarray(a, dtype=np.float32) for a in (q, k, v, Wq, bq, Wo, bo)
    )
    nc = _get(1, "full", DEFAULT_VARIANT)
    in_maps = make_in_maps(q, k, v, Wq, bq, Wo, bo, DEFAULT_VARIANT)
    B = q.shape[0]
    out = np.empty((B, S, D), dtype=np.float32)
    for attempt in range(3):
        res = run_bass_kernel_spmd(nc, in_maps, list(range(8)))
        for b in range(B):
            out[b] = (res.results[2 * b]["out"]
                      + res.results[2 * b + 1]["out"] + bo)
        # rare transient (axon/HW flake) can surface as NaNs; rerun
        if np.isfinite(out).all():
            break
    return out

